# revision 14
# baseline (speedup 1.0000x reference)
"""GCN (4x GCNConv + global_add_pool + MLP) on 8 Trainium2 NeuronCores.

Sharding: nodes partitioned into 8 contiguous blocks of 12544 (dst
partitioning); each edge lives on the core that owns its dst node, so the
scatter-add is core-local.  The gather side reads from a replicated per-layer
table built with one AllGather per layer (bf16, rows padded to 128 cols =
256B so the token-DMA gather can fetch them).

Edge messages are fetched with gpsimd.dma_gather (token gather): int16
indices force a 4-way quadrant split of the table (32768 rows each); edge
slots are grouped (tile, quadrant)-major and padded to 128-slot batches with
dummy index 0 — the pure-0/1 one-hot (doff=-1 on pad slots) zeroes their
contribution in the scatter matmul.

Numerics: the GCN norm dinv[src]*dinv[dst] is applied as two per-node f32
scales (scalar-engine activation scale on the table rows; vector stt on the
scatter accumulator), the one-hot is exact 0/1 bf16, and all weight matmuls
use hi/lo bf16 pairs, so the only bf16 roundings are h/table storage.
"""

import math

import numpy as np
import ml_dtypes

BF16 = ml_dtypes.bfloat16

P = 128          # partitions / slot-batch size / node-tile size
NFEAT = 32
HID = 96
HP = 128         # padded table row width (256B in bf16)
NG = 2048        # graphs
NCORES = 8
NPC = 12544      # nodes per core (98 * 128)
NT = NPC // P    # 98 node tiles per core
NPAD = NPC * NCORES
NG_PAD = NG + 512
QROW = 32768     # table quadrant height (int16 index range)
NQ = 4
KB = 4           # tiles per gather block


def _hi_lo(w):
    w = np.asarray(w, np.float32)
    hi = w.astype(BF16)
    lo = (w - hi.astype(np.float32)).astype(BF16)
    return hi, lo


def _make_blocks(B):
    """Partition tiles into ceil(NT/KB) bins, greedily balancing total
    column count per bin (deterministic, shared across cores)."""
    nb = (NT + KB - 1) // KB
    mt = [(int(B[t].sum()), t) for t in range(NT)]
    mt.sort(key=lambda x: (-x[0], x[1]))
    bins = [[] for _ in range(nb)]
    loads = [0] * nb
    for w, t in mt:
        i = min(range(nb), key=lambda k: (loads[k], len(bins[k]), k))
        if len(bins[i]) >= KB:
            cands = [k for k in range(nb) if len(bins[k]) < KB]
            i = min(cands, key=lambda k: (loads[k], k))
        bins[i].append(t)
        loads[i] += w
    return [sorted(b) for b in bins if b]


# ----------------------------------------------------------------------------
# Host-side preprocessing.
# ----------------------------------------------------------------------------

def _prep(x, edge_index, batch):
    N = x.shape[0]
    src = np.asarray(edge_index[0], dtype=np.int64)
    dst = np.asarray(edge_index[1], dtype=np.int64)
    loops = np.arange(N, dtype=np.int64)
    src = np.concatenate([src, loops])
    dst = np.concatenate([dst, loops])
    deg = np.bincount(dst, minlength=N).astype(np.float64)
    dinv = np.where(deg > 0, 1.0 / np.sqrt(np.maximum(deg, 1.0)), 0.0).astype(
        np.float32
    )
    batch = np.asarray(batch, dtype=np.int64)

    owner = dst // NPC
    # per core, per tile, per quadrant: (idx_rel int16 list, doff list)
    edges = []   # [core][tile][quad] -> (idx_rel, doff)
    for c in range(NCORES):
        m = owner == c
        s_c = src[m]
        d_c = dst[m] - c * NPC
        o = np.argsort(d_c, kind="stable")
        s_c, d_c = s_c[o], d_c[o]
        t_c = d_c // P
        percore = []
        for t in range(NT):
            sel = t_c == t
            s_t = s_c[sel]
            d_t = (d_c[sel] - t * P).astype(np.float32)
            q_t = s_t // QROW
            perq = []
            for q in range(NQ):
                qs = q_t == q
                perq.append(((s_t[qs] - q * QROW).astype(np.int16), d_t[qs]))
            percore.append(perq)
        edges.append(percore)

    # shared (max over cores) slot-batch counts per (tile, quadrant)
    B = np.zeros((NT, NQ), np.int64)
    for t in range(NT):
        for q in range(NQ):
            mx = max(len(edges[c][t][q][0]) for c in range(NCORES))
            B[t, q] = (mx + P - 1) // P

    # graph-tile count
    GT = 1
    for c in range(NCORES):
        n0 = c * NPC
        nreal = min(NPC, max(0, N - n0))
        if nreal > 0:
            gb = int(batch[n0])
            gmax = int(batch[n0 + nreal - 1])
            GT = max(GT, int(math.ceil((gmax - gb + 1) / P)))
    assert NG + GT * P <= NG_PAD + P

    blocks = _make_blocks(B)
    CT = int(B.sum())              # total msg/doff columns per layer
    IDXC = CT * P // 16            # idx columns

    inputs = []
    for c in range(NCORES):
        idx_arr = np.zeros((P, IDXC), np.int16)
        doff_flat = np.full(CT * P, -1.0, np.float32)
        coff = 0
        gcol = 0
        for blk in blocks:
            for q in range(NQ):
                for t in blk:
                    nb = int(B[t, q]) * P
                    if nb == 0:
                        continue
                    idx_t, doff_t = edges[c][t][q]
                    k = len(idx_t)
                    ilist = np.zeros(nb, np.int16)
                    ilist[:k] = idx_t
                    w = ilist.reshape(nb // 16, 16).T     # [16, nb/16]
                    for g8 in range(8):
                        idx_arr[16 * g8 : 16 * (g8 + 1),
                                coff : coff + nb // 16] = w
                    doff_flat[gcol * P : gcol * P + k] = doff_t
                    coff += nb // 16
                    gcol += int(B[t, q])
        assert coff == IDXC and gcol == CT
        doff_arr = doff_flat.reshape(CT, P).T.copy()      # [P, CT]

        n0 = c * NPC
        nreal = min(NPC, max(0, N - n0))
        xs = np.zeros((NPC, NFEAT), np.float32)
        xs[:nreal] = np.asarray(x, np.float32)[n0 : n0 + nreal]
        dloc = np.zeros(NPC, np.float32)
        dloc[:nreal] = dinv[n0 : n0 + nreal]
        gbase = int(batch[n0]) if nreal > 0 else 0
        pg = np.full(NPC, 1.0e9, np.float32)
        pg[:nreal] = (batch[n0 : n0 + nreal] - gbase).astype(np.float32)
        pgt = pg.reshape(NT, P).T
        pgg = np.zeros((P, GT * NT), np.float32)
        for g in range(GT):
            pgg[:, g * NT : (g + 1) * NT] = pgt - g * P
        growidx = (
            gbase
            + np.arange(GT, dtype=np.int32)[None, :] * P
            + np.arange(P, dtype=np.int32)[:, None]
        ).astype(np.int32)

        inputs.append(
            dict(
                xt=np.ascontiguousarray(xs.T).astype(BF16),
                idx=idx_arr,
                doff=doff_arr,
                dinv=np.ascontiguousarray(dloc.reshape(NT, P).T),
                pgg=pgg,
                growidx=growidx,
            )
        )
    return inputs, B, GT


# ----------------------------------------------------------------------------
# Numpy emulation (validates the index plumbing of the device flow).
# ----------------------------------------------------------------------------

def _emulate(inputs, B, GT, weights):
    blocks = _make_blocks(B)
    tables = [None] * NCORES
    h = [inp["xt"].astype(np.float32).T for inp in inputs]  # [NPC, K]
    Ws = [weights["W1"], weights["W2"], weights["W3"], weights["W4"]]
    bs = [weights["b1"], weights["b2"], weights["b3"], weights["b4"]]
    iota = np.arange(P, dtype=np.float32)
    for l in range(4):
        for c in range(NCORES):
            dv = inputs[c]["dinv"].T.reshape(NPC, 1)
            tables[c] = (h[c] @ Ws[l]) * dv
        table = np.concatenate(tables, axis=0)  # [NPAD, HID]
        for c in range(NCORES):
            inp = inputs[c]
            idx = inp["idx"]
            hn = np.zeros((NPC, HID), np.float32)
            coff = 0
            gcol = 0
            for blk in blocks:
                acc = {t: np.zeros((P, HID), np.float32) for t in blk}
                for q in range(NQ):
                    for t in blk:
                        nb = int(B[t, q]) * P
                        if nb == 0:
                            continue
                        ilist = idx[:16, coff : coff + nb // 16].T.reshape(-1)
                        rows = table[ilist.astype(np.int64) + q * QROW]
                        for cc in range(nb // P):
                            sl = slice(cc * P, (cc + 1) * P)
                            msg = rows[sl]                    # [P(slot), HID]
                            dof = inp["doff"][:, gcol + cc]   # [P]
                            oh = (iota[None, :] == dof[:, None]).astype(
                                np.float32
                            )  # [slot, dst]
                            acc[t] += oh.T @ msg
                        coff += nb // 16
                        gcol += int(B[t, q])
                for t in blk:
                    dv2 = inp["dinv"][:, t : t + 1]
                    hn[t * P : (t + 1) * P] = np.maximum(
                        acc[t] * dv2 + np.asarray(bs[l]).reshape(1, HID), 0.0
                    )
            h[c] = hn
    gsum = np.zeros((NG_PAD, HID), np.float32)
    for c in range(NCORES):
        inp = inputs[c]
        for g in range(GT):
            acc = np.zeros((P, HID), np.float32)
            for t in range(NT):
                oh = (
                    iota[None, :] == inp["pgg"][:, g * NT + t : g * NT + t + 1]
                ).astype(np.float32)
                acc += oh.T @ h[c][t * P : (t + 1) * P]
            gsum[inp["growidx"][:, g]] += acc
    g = gsum[:NG]
    z = np.maximum(g @ weights["Wf1"] + weights["bf1"][None, :], 0.0)
    return z @ weights["Wf2"] + weights["bf2"]


# ----------------------------------------------------------------------------
# Bass program.
# ----------------------------------------------------------------------------

def _build_program(B, GT, bf2val):
    from concourse import bacc, bass, mybir, tile
    from concourse.bass import AP

    f32 = mybir.dt.float32
    bf16 = mybir.dt.bfloat16
    i32 = mybir.dt.int32
    i16 = mybir.dt.int16
    AF = mybir.ActivationFunctionType
    OP = mybir.AluOpType

    blocks = _make_blocks(B)
    CT = int(B.sum())
    IDXC = CT * P // 16
    CB = [int(sum(B[t, q] for t in blk for q in range(NQ))) for blk in blocks]
    CBMAX = max(CB)

    nc = bacc.Bacc("TRN2", target_bir_lowering=False, debug=False)

    xt_p = nc.declare_dram_parameter("xt", [NFEAT, NPC], bf16, isOutput=False)
    idx_p = nc.declare_dram_parameter("idx", [P, IDXC], i16, isOutput=False)
    doff_p = nc.declare_dram_parameter("doff", [P, CT], f32, isOutput=False)
    dinv_p = nc.declare_dram_parameter("dinv", [P, NT], f32, isOutput=False)
    pgg_p = nc.declare_dram_parameter("pgg", [P, GT * NT], f32, isOutput=False)
    grow_p = nc.declare_dram_parameter("growidx", [P, GT], i32, isOutput=False)
    w_hi_ps, w_lo_ps = [], []
    for l in range(4):
        K = NFEAT if l == 0 else HID
        w_hi_ps.append(
            nc.declare_dram_parameter(f"w{l + 1}h", [K, HP], bf16, isOutput=False)
        )
        w_lo_ps.append(
            nc.declare_dram_parameter(f"w{l + 1}l", [K, HP], bf16, isOutput=False)
        )
    b_ps = [
        nc.declare_dram_parameter(f"b{l + 1}", [1, HID], f32, isOutput=False)
        for l in range(4)
    ]
    wf1h_p = nc.declare_dram_parameter("wf1h", [HID, 32], bf16, isOutput=False)
    wf1l_p = nc.declare_dram_parameter("wf1l", [HID, 32], bf16, isOutput=False)
    bf1_p = nc.declare_dram_parameter("bf1", [32, 1], f32, isOutput=False)
    wf2h_p = nc.declare_dram_parameter("wf2h", [32, 1], bf16, isOutput=False)
    wf2l_p = nc.declare_dram_parameter("wf2l", [32, 1], bf16, isOutput=False)
    out_p = nc.declare_dram_parameter("out", [1, NG], f32, isOutput=True)

    groups = [list(range(NCORES))]

    with tile.TileContext(nc) as tc:
        with (
            tc.tile_pool(name="const", bufs=1) as cp,
            tc.tile_pool(name="sb", bufs=1) as sb,
            tc.tile_pool(name="ps", bufs=2, space="PSUM") as ps,
            tc.tile_pool(name="dram", bufs=1, space="DRAM") as dp,
        ):
            hnm = cp.tile([P, NT, HID], bf16)
            idx_sb = cp.tile([P, IDXC], i16)
            doff_sb = cp.tile([P, CT], f32)
            dinv_sb = cp.tile([P, NT], f32)
            pgg_sb = cp.tile([P, GT * NT], f32)
            grow_sb = cp.tile([P, GT], i32)
            iota_i = cp.tile([P, P], i32)
            iota_f = cp.tile([P, P], f32)
            ident_b = cp.tile([P, P], bf16)
            ident_f = cp.tile([P, P], f32)
            ones1 = cp.tile([1, P], f32)
            zero_sb = cp.tile([P, HID], f32)
            w_hi_sb, w_lo_sb = [], []
            for l in range(4):
                K = NFEAT if l == 0 else HID
                w_hi_sb.append(cp.tile([K, HP], bf16, name=f"wh{l}"))
                w_lo_sb.append(cp.tile([K, HP], bf16, name=f"wl{l}"))
            brow = [cp.tile([1, HID], f32, name=f"brow{l}") for l in range(4)]
            bias_bc = [cp.tile([P, HID], f32, name=f"bias{l}") for l in range(4)]
            wf1h_sb = cp.tile([HID, 32], bf16)
            wf1l_sb = cp.tile([HID, 32], bf16)
            bf1_sb = cp.tile([32, 1], f32)
            wf2h_sb = cp.tile([32, 1], bf16)
            wf2l_sb = cp.tile([32, 1], bf16)

            contrib = [
                dp.tile([NPC, HP], bf16, name=f"contrib{l}", bufs=1)
                for l in range(4)
            ]
            table = [
                dp.tile([NPAD, HP], bf16, name=f"table{l}", bufs=1,
                        addr_space="Shared")
                for l in range(4)
            ]
            gin = dp.tile([NG_PAD, HID], f32)
            gout = dp.tile([NG_PAD, HID], f32, addr_space="Shared")

            nc.sync.dma_start(out=idx_sb[:], in_=idx_p[:])
            nc.sync.dma_start(out=doff_sb[:], in_=doff_p[:])
            nc.sync.dma_start(out=dinv_sb[:], in_=dinv_p[:])
            nc.sync.dma_start(out=pgg_sb[:], in_=pgg_p[:])
            nc.sync.dma_start(out=grow_sb[:], in_=grow_p[:])
            for l in range(4):
                nc.sync.dma_start(out=w_hi_sb[l][:], in_=w_hi_ps[l][:])
                nc.sync.dma_start(out=w_lo_sb[l][:], in_=w_lo_ps[l][:])
                nc.sync.dma_start(out=brow[l][:], in_=b_ps[l][:])
            nc.sync.dma_start(out=wf1h_sb[:], in_=wf1h_p[:])
            nc.sync.dma_start(out=wf1l_sb[:], in_=wf1l_p[:])
            nc.sync.dma_start(out=bf1_sb[:], in_=bf1_p[:])
            nc.sync.dma_start(out=wf2h_sb[:], in_=wf2h_p[:])
            nc.sync.dma_start(out=wf2l_sb[:], in_=wf2l_p[:])

            from concourse.masks import make_identity

            make_identity(nc, ident_b[:])
            make_identity(nc, ident_f[:])
            nc.gpsimd.iota(
                iota_i[:], pattern=[[1, P]], base=0, channel_multiplier=0
            )
            nc.vector.tensor_copy(out=iota_f[:], in_=iota_i[:])
            nc.vector.memset(ones1[:], 1.0)
            nc.vector.memset(zero_sb[:], 0.0)

            for l in range(4):
                pb = ps.tile([P, HID], f32, tag="xw")
                nc.tensor.matmul(
                    out=pb[:], lhsT=ones1[:], rhs=brow[l][:], start=True,
                    stop=True,
                )
                nc.vector.tensor_copy(out=bias_bc[l][:], in_=pb[:])

            for r in range(NG_PAD // P):
                nc.sync.dma_start(
                    out=gin[r * P : (r + 1) * P, :], in_=zero_sb[:]
                )

            # ---- 4 GCN layers ------------------------------------------
            for l in range(4):
                with nc.named_scope(f"build{l}"):
                    for t in range(NT):
                        if l == 0:
                            hTt_sb = sb.tile([NFEAT, P], bf16, tag="xtt", bufs=3)
                            nc.sync.dma_start(
                                out=hTt_sb[:], in_=xt_p[:, t * P : (t + 1) * P]
                            )
                        else:
                            ptr = ps.tile([HID, P], bf16, tag="tr")
                            nc.tensor.transpose(
                                out=ptr[:], in_=hnm[:, t, :],
                                identity=ident_b[:],
                            )
                            hTt_sb = sb.tile([HID, P], bf16, tag="hTt", bufs=3)
                            nc.vector.tensor_copy(out=hTt_sb[:], in_=ptr[:])
                        pxw = ps.tile([P, HP], f32, tag="xw")
                        nc.tensor.matmul(
                            out=pxw[:], lhsT=hTt_sb[:], rhs=w_hi_sb[l][:],
                            start=True, stop=False,
                        )
                        nc.tensor.matmul(
                            out=pxw[:], lhsT=hTt_sb[:], rhs=w_lo_sb[l][:],
                            start=False, stop=True,
                        )
                        ctile = sb.tile([P, HP], bf16, tag="ct", bufs=3)
                        nc.scalar.activation(
                            out=ctile[:], in_=pxw[:], func=AF.Identity,
                            scale=dinv_sb[:, t : t + 1],
                        )
                        nc.sync.dma_start(
                            out=contrib[l][t * P : (t + 1) * P, :], in_=ctile[:]
                        )
                with nc.named_scope(f"ag{l}"):
                    nc.gpsimd.collective_compute(
                        "AllGather",
                        mybir.AluOpType.bypass,
                        replica_groups=groups,
                        ins=[contrib[l][:]],
                        outs=[table[l][:]],
                    )
                with nc.named_scope(f"mp{l}"):
                    coff = 0
                    gcol = 0
                    for bi, blk in enumerate(blocks):
                        cb = CB[bi]
                        msgB = sb.tile([P, CBMAX * HP], bf16, tag="msg", bufs=2)
                        msg3 = msgB[:].rearrange("p (c e) -> p c e", e=HP)
                        local = 0
                        for q in range(NQ):
                            nbq = int(sum(B[t, q] for t in blk)) * P
                            if nbq == 0:
                                continue
                            r0 = q * QROW
                            r1 = min(r0 + QROW, NPAD)
                            nc.gpsimd.dma_gather(
                                out_ap=msg3[:, local : local + nbq // P, :],
                                in_ap=table[l][r0:r1, :],
                                idxs_ap=idx_sb[:, coff : coff + nbq // 16],
                                num_idxs=nbq,
                                num_idxs_reg=nbq,
                                elem_size=HP,
                                single_packet=False,
                            )
                            coff += nbq // 16
                            local += nbq // P
                        ohB = sb.tile([P, CBMAX * P], bf16, tag="oh", bufs=2)
                        dsl = doff_sb[:, gcol : gcol + cb]
                        a0 = AP(iota_f[:].tensor, iota_f[:].offset,
                                [iota_f[:].ap[0], [0, cb], iota_f[:].ap[1]])
                        a1 = AP(dsl.tensor, dsl.offset,
                                [dsl.ap[0], dsl.ap[1], [0, P]])
                        o = AP(ohB[:].tensor, ohB[:].offset,
                               [ohB[:].ap[0], [P, cb], [1, P]])
                        nc.vector.tensor_tensor(
                            out=o, in0=a0, in1=a1, op=OP.is_equal
                        )
                        # per tile: accumulate its (q-scattered) columns
                        col_of = {}
                        lc = 0
                        for q in range(NQ):
                            for t in blk:
                                for _ in range(int(B[t, q])):
                                    col_of.setdefault(t, []).append(lc)
                                    lc += 1
                        for t in blk:
                            cols = col_of.get(t, [])
                            if not cols:
                                continue
                            pacc = ps.tile([P, HID], f32, tag="acc")
                            for j, c in enumerate(cols):
                                nc.tensor.matmul(
                                    out=pacc[:],
                                    lhsT=ohB[:, c * P : (c + 1) * P],
                                    rhs=msgB[:, c * HP : c * HP + HID],
                                    start=(j == 0),
                                    stop=(j == len(cols) - 1),
                                )
                            hb = sb.tile([P, HID], f32, tag="hb", bufs=3)
                            nc.vector.scalar_tensor_tensor(
                                out=hb[:], in0=pacc[:],
                                scalar=dinv_sb[:, t : t + 1],
                                in1=bias_bc[l][:],
                                op0=OP.mult, op1=OP.add,
                            )
                            nc.scalar.activation(
                                out=hnm[:, t, :], in_=hb[:], func=AF.Relu,
                            )
                        gcol += cb

            # ---- global_add_pool ---------------------------------------
            with nc.named_scope("pool"):
                PCH = 14
                for g in range(GT):
                    pg_ps = ps.tile([P, HID], f32, tag="acc")
                    for t0 in range(0, NT, PCH):
                        tn = min(PCH, NT - t0)
                        ohp = sb.tile([P, PCH * P], bf16, tag="ohp", bufs=2)
                        dsl = pgg_sb[:, g * NT + t0 : g * NT + t0 + tn]
                        a0 = AP(iota_f[:].tensor, iota_f[:].offset,
                                [iota_f[:].ap[0], [0, tn], iota_f[:].ap[1]])
                        a1 = AP(dsl.tensor, dsl.offset,
                                [dsl.ap[0], dsl.ap[1], [0, P]])
                        o = AP(ohp[:].tensor, ohp[:].offset,
                               [ohp[:].ap[0], [P, tn], [1, P]])
                        nc.vector.tensor_tensor(
                            out=o, in0=a0, in1=a1, op=OP.is_equal
                        )
                        for tt2 in range(tn):
                            t = t0 + tt2
                            nc.tensor.matmul(
                                out=pg_ps[:],
                                lhsT=ohp[:, tt2 * P : (tt2 + 1) * P],
                                rhs=hnm[:, t, :],
                                start=(t == 0),
                                stop=(t == NT - 1),
                            )
                    gsb = sb.tile([P, HID], f32, tag="gsb", bufs=2)
                    nc.vector.tensor_copy(out=gsb[:], in_=pg_ps[:])
                    nc.gpsimd.indirect_dma_start(
                        out=gin[:],
                        out_offset=bass.IndirectOffsetOnAxis(
                            ap=grow_sb[:, g : g + 1], axis=0
                        ),
                        in_=gsb[:],
                        in_offset=None,
                    )

                nc.gpsimd.collective_compute(
                    "AllReduce",
                    mybir.AluOpType.add,
                    replica_groups=groups,
                    ins=[gin[:]],
                    outs=[gout[:]],
                )

            # ---- MLP head (hi/lo bf16, chunked) ------------------------
            with nc.named_scope("head"):
                for j in range(NG // 512):
                    gT = sb.tile([HID, 512], f32, tag="gT", bufs=2)
                    for jj in range(4):
                        r0 = j * 512 + jj * P
                        grow_t = sb.tile([P, HID], f32, tag="gsb", bufs=2)
                        nc.sync.dma_start(
                            out=grow_t[:], in_=gout[r0 : r0 + P, :]
                        )
                        pT = ps.tile([HID, P], f32, tag="tr2")
                        nc.tensor.transpose(
                            out=pT[:], in_=grow_t[:], identity=ident_f[:]
                        )
                        nc.vector.tensor_copy(
                            out=gT[:, jj * P : (jj + 1) * P], in_=pT[:]
                        )
                    g_hi = sb.tile([HID, 512], bf16, tag="ghi", bufs=2)
                    g_lo = sb.tile([HID, 512], bf16, tag="glo", bufs=2)
                    g_tmp = sb.tile([HID, 512], f32, tag="gtmp", bufs=2)
                    nc.vector.tensor_copy(out=g_hi[:], in_=gT[:])
                    nc.vector.tensor_copy(out=g_tmp[:], in_=g_hi[:])
                    nc.vector.tensor_sub(out=g_tmp[:], in0=gT[:], in1=g_tmp[:])
                    nc.vector.tensor_copy(out=g_lo[:], in_=g_tmp[:])
                    pz = ps.tile([32, 512], f32, tag="xw")
                    nc.tensor.matmul(out=pz[:], lhsT=wf1h_sb[:],
                                     rhs=g_hi[:], start=True, stop=False)
                    nc.tensor.matmul(out=pz[:], lhsT=wf1l_sb[:],
                                     rhs=g_hi[:], start=False, stop=False)
                    nc.tensor.matmul(out=pz[:], lhsT=wf1h_sb[:],
                                     rhs=g_lo[:], start=False, stop=True)
                    zT = sb.tile([32, 512], f32, tag="zT", bufs=2)
                    nc.scalar.activation(
                        out=zT[:], in_=pz[:], func=AF.Relu,
                        bias=bf1_sb[:, 0:1], scale=1.0,
                    )
                    z_hi = sb.tile([32, 512], bf16, tag="zhi", bufs=2)
                    z_lo = sb.tile([32, 512], bf16, tag="zlo", bufs=2)
                    z_tmp = sb.tile([32, 512], f32, tag="ztmp", bufs=2)
                    nc.vector.tensor_copy(out=z_hi[:], in_=zT[:])
                    nc.vector.tensor_copy(out=z_tmp[:], in_=z_hi[:])
                    nc.vector.tensor_sub(out=z_tmp[:], in0=zT[:], in1=z_tmp[:])
                    nc.vector.tensor_copy(out=z_lo[:], in_=z_tmp[:])
                    po = ps.tile([1, 512], f32, tag="tr2")
                    nc.tensor.matmul(out=po[:], lhsT=wf2h_sb[:],
                                     rhs=z_hi[:], start=True, stop=False)
                    nc.tensor.matmul(out=po[:], lhsT=wf2l_sb[:],
                                     rhs=z_hi[:], start=False, stop=False)
                    nc.tensor.matmul(out=po[:], lhsT=wf2h_sb[:],
                                     rhs=z_lo[:], start=False, stop=True)
                    och = sb.tile([1, 512], f32, tag="och", bufs=2)
                    nc.vector.tensor_scalar(
                        out=och[:], in0=po[:],
                        scalar1=float(bf2val), scalar2=None, op0=OP.add,
                    )
                    nc.sync.dma_start(
                        out=out_p[:, j * 512 : (j + 1) * 512], in_=och[:]
                    )

    nc.finalize()
    return nc


# ----------------------------------------------------------------------------
# Entry point.
# ----------------------------------------------------------------------------

_RUN_KWARGS = {}
_LAST_RESULT = None


def kernel(
    x,
    edge_index,
    batch,
    W1,
    b1,
    W2,
    b2,
    W3,
    b3,
    W4,
    b4,
    Wf1,
    bf1,
    Wf2,
    bf2,
):
    from concourse.bass_utils import run_bass_kernel_spmd

    inputs, B, GT = _prep(np.asarray(x), np.asarray(edge_index), np.asarray(batch))
    bf2val = float(np.asarray(bf2).reshape(-1)[0])
    nc = _build_program(B, GT, bf2val)

    shared = {}
    for name, w in (("w1", W1), ("w2", W2), ("w3", W3), ("w4", W4)):
        wp = np.zeros((np.asarray(w).shape[0], HP), np.float32)
        wp[:, :HID] = np.asarray(w, np.float32)
        hi, lo = _hi_lo(wp)
        shared[name + "h"] = hi
        shared[name + "l"] = lo
    for name, w, shape in (
        ("wf1", Wf1, (HID, 32)),
        ("wf2", Wf2, (32, 1)),
    ):
        hi, lo = _hi_lo(np.asarray(w, np.float32).reshape(shape))
        shared[name + "h"] = hi
        shared[name + "l"] = lo
    shared.update(
        b1=np.asarray(b1, np.float32).reshape(1, HID),
        b2=np.asarray(b2, np.float32).reshape(1, HID),
        b3=np.asarray(b3, np.float32).reshape(1, HID),
        b4=np.asarray(b4, np.float32).reshape(1, HID),
        bf1=np.asarray(bf1, np.float32).reshape(32, 1),
    )
    in_maps = [{**inputs[c], **shared} for c in range(NCORES)]
    res = run_bass_kernel_spmd(
        nc, in_maps, core_ids=list(range(NCORES)), **_RUN_KWARGS
    )
    global _LAST_RESULT
    _LAST_RESULT = res
    out = np.asarray(res.results[0]["out"]).reshape(NG, 1).astype(np.float32)
    return out
